# revision 16
# baseline (speedup 1.0000x reference)
"""Trainium2 Bass kernel for nn_Mixture_Loss_74053826118054.

Strategy (valid-row compaction + data parallel over 8 cores):
  Every term of the loss depends only on 5 per-(s,b)-row reductions over D:
    ll = sum_d l^2,  tt = sum_d t^2,  lt = sum_d l*t,
    ln = sum_d l[s]*l[s+1]  (consecutive sentences, same batch),
    tn = sum_d t[s]*t[s+1]
  The padding mask is known on the host, and every quantity is only ever
  USED on valid rows (~50%): masked MSE / cos / deltas all multiply by the
  valid mask, and ln/tn are only used on valid PAIRS (both rows valid,
  ~25%). The host packs only the valid rows, batch-major with maximal
  consecutive-valid runs kept contiguous, and ships the packed array:
  half the HBM traffic and half the compute of the dense kernel.

  Rows in runs of length >= 2 ("run region") need all five quantities;
  isolated valid rows ("iso region", no valid neighbor) need only
  ll/tt/lt. The packed list is [run rows | iso rows]; overflow iso rows
  spill into the run region's spare slots (their pair products are
  garbage the host ignores). Pair validity is re-derived on the host
  from the packed (b, s) list, so any garbage pair (run boundaries,
  spilled iso rows, zero padding) is dropped exactly.

Device layout per core: partitions hold RG consecutive packed rows
(+1 overlap slot = next partition's/core's first row, so consecutive-row
products are free-axis slices) plus IG iso rows. l and t of one row are
interleaved in DRAM ([row, {l,t}, D]) so each chunk load is one 8 KB
descriptor per partition. Per chunk: ACT does the two squares with fused
accumulate; DVE does lt/ln/tn as scalar_tensor_tensor with fused
accumulate. No GpSimd: its SBUF port is shared with DVE ("POOL slot")
and concurrent GpSimd ops double DVE 2-port op latency.

DMA schedule: all chunk loads are issued back-to-back on the Sync
engine's HWDGE ring; one ring = FIFO completion, so a single monotonic
semaphore (16 incs per load) gates the compute pipeline. The overlap
slot is NOT re-read from HBM: ACT's ring copies it SBUF->SBUF with a
one-partition shift (plus one 8 KB HBM edge row for partition 127).
Each compute engine stores its own merged result block on its own ring
right after draining, so the only cross-engine sync is the final
out_sem wait. The tiny O(S*B) finish (cos, deltas, rank-compaction,
delta-of-delta) runs on host in float64, reproducing the reference
semantics exactly.
"""

import numpy as np

from contextlib import ExitStack

import concourse.bass as bass
import concourse.mybir as mybir
from concourse.bass_utils import run_bass_kernel_spmd

F32 = mybir.dt.float32
AF = mybir.ActivationFunctionType
ALU = mybir.AluOpType

N_CORES = 8
S, B, D = 64, 256, 1024
P = 128

_cached = {}


def _build_program(RG, IG):
    """Bass program for RG run-chunks (+1 overlap slot) and IG iso-chunks."""
    key = (RG, IG)
    if key in _cached:
        return _cached[key]
    NSLOT = RG + IG + 1          # run slots + iso slots + overlap (last)
    OV = RG + IG                 # overlap slot index
    NC = RG + IG                 # result columns for ll/tt/lt
    nc = bass.Bass()
    # slot-major, exactly the SBUF layout: x[slot, partition] = [l | t]
    # of that slot's row -> every chunk load is one contiguous 1MB region
    x_in = nc.dram_tensor("x", [NSLOT, P, 2, D], F32, kind="ExternalInput")
    # merged outputs: one store per compute engine
    out_act = nc.dram_tensor("act", [P, 2 * NC], F32, kind="ExternalOutput")
    out_dve = nc.dram_tensor("dve", [P, NC + 2 * RG], F32,
                             kind="ExternalOutput")

    with ExitStack() as stack:
        ec = stack.enter_context
        # one semaphore per chunk: DMA completions are only FIFO per SDMA
        # engine, so a summed counter can pass with a mix of chunks
        csem = [ec(nc.semaphore(f"c{i}")) for i in range(NSLOT)]
        out_sem = ec(nc.semaphore("out"))
        dve_done = ec(nc.semaphore("dvedone"))
        xbig = ec(nc.sbuf_tensor([P, NSLOT * 2 * D], F32))
        dummies = ec(nc.sbuf_tensor([P, 8], F32))
        ract = ec(nc.sbuf_tensor([P, 2 * NC], F32))       # [ll | tt]
        rdve = ec(nc.sbuf_tensor([P, NC + 2 * RG], F32))  # [lt | ln | tn]
        block = ec(nc.Block())
        xc = xbig.ap().rearrange("p (c v d) -> p c v d", v=2, d=D)

        def chunk(slot, half):
            return xc[:, slot, half, :]

        def bcast(k):
            return dummies.ap()[:, k:k + 1].broadcast_to((P, D))

        rll = ract.ap()[:, 0:NC]
        rtt = ract.ap()[:, NC:2 * NC]
        rlt = rdve.ap()[:, 0:NC]
        rln = rdve.ap()[:, NC:NC + RG]
        rtn = rdve.ap()[:, NC + RG:NC + 2 * RG]

        @block.sync
        def _(sync):
            # one FIFO ring, consumption order, overlap slot last
            for s in range(NSLOT):
                sync.dma_start(out=xc[:, s, :, :],
                               in_=x_in[s]).then_inc(csem[s], 16)
            sync.wait_ge(dve_done, 1)
            sync.dma_start(out=out_dve[:], in_=rdve.ap()).then_inc(
                out_sem, 16)
            sync.wait_ge(out_sem, 32)

        @block.scalar
        def _(scalar):
            for col in range(NC):        # run slots then iso slots = cols
                scalar.wait_ge(csem[col], 16)
                scalar.activation(bcast(0), chunk(col, 0), AF.Square,
                                  accum_out=rll[:, col:col + 1])
                scalar.activation(bcast(1), chunk(col, 1), AF.Square,
                                  accum_out=rtt[:, col:col + 1])
            scalar.drain()
            scalar.dma_start(out=out_act[:], in_=ract.ap()).then_inc(
                out_sem, 16)

        @block.vector
        def _(vector):
            for j in range(RG):
                vector.wait_ge(csem[j], 16)
                vector.scalar_tensor_tensor(
                    out=bcast(2), in0=chunk(j, 0), scalar=0.0,
                    in1=chunk(j, 1), op0=ALU.bypass, op1=ALU.mult,
                    accum_out=rlt[:, j:j + 1])
                if j < RG - 1:           # j = RG-1 pairs with the overlap
                    vector.wait_ge(csem[j + 1], 16)   # slot, loaded last
                    vector.scalar_tensor_tensor(
                        out=bcast(3), in0=chunk(j, 0), scalar=0.0,
                        in1=chunk(j + 1, 0), op0=ALU.bypass, op1=ALU.mult,
                        accum_out=rln[:, j:j + 1])
                    vector.scalar_tensor_tensor(
                        out=bcast(4), in0=chunk(j, 1), scalar=0.0,
                        in1=chunk(j + 1, 1), op0=ALU.bypass, op1=ALU.mult,
                        accum_out=rtn[:, j:j + 1])
            for k in range(IG):
                slot = RG + k
                vector.wait_ge(csem[slot], 16)
                vector.scalar_tensor_tensor(
                    out=bcast(2), in0=chunk(slot, 0), scalar=0.0,
                    in1=chunk(slot, 1), op0=ALU.bypass, op1=ALU.mult,
                    accum_out=rlt[:, slot:slot + 1])
            vector.wait_ge(csem[OV], 16)
            vector.scalar_tensor_tensor(
                out=bcast(3), in0=chunk(RG - 1, 0), scalar=0.0,
                in1=chunk(OV, 0), op0=ALU.bypass, op1=ALU.mult,
                accum_out=rln[:, RG - 1:RG])
            vector.scalar_tensor_tensor(
                out=bcast(4), in0=chunk(RG - 1, 1), scalar=0.0,
                in1=chunk(OV, 1), op0=ALU.bypass, op1=ALU.mult,
                accum_out=rtn[:, RG - 1:RG])
            vector.drain().then_inc(dve_done, 1)

    _cached[key] = nc
    return nc


def _plan(mask):
    """Packed-row plan from the padding mask.

    Returns (bs_b, bs_s, pair_ok, RG, IG): packed order = all rows of
    runs (length >= 2, batch-major, runs contiguous) then isolated valid
    rows. pair_ok[g] marks packed-adjacent pairs (g, g+1) that are true
    consecutive same-batch valid pairs.
    """
    valid = ~mask                                   # (B, S)
    left = np.zeros_like(valid)
    left[:, 1:] = valid[:, :-1]
    right = np.zeros_like(valid)
    right[:, :-1] = valid[:, 1:]
    iso = valid & ~left & ~right
    runm = valid & ~iso
    rb, rs = np.nonzero(runm)                       # lexicographic: runs stay
    ib, is_ = np.nonzero(iso)                       # contiguous in order
    bs_b = np.concatenate([rb, ib])
    bs_s = np.concatenate([rs, is_])
    n_run = len(rb)
    tot = len(bs_b)
    pair_ok = (bs_b[:-1] == bs_b[1:]) & (bs_s[:-1] + 1 == bs_s[1:])
    RG = max(1, -(-n_run // (N_CORES * P)))
    IG = max(0, -(-(tot - N_CORES * P * RG) // (N_CORES * P)))
    return bs_b, bs_s, pair_ok, RG, IG


def _run_device(logits, tgt_out, plan, trace=False):
    bs_b, bs_s, pair_ok, RG, IG = plan
    nc = _build_program(RG, IG)
    NC = RG + IG
    tot = len(bs_b)

    lf = np.swapaxes(logits, 0, 1)                  # (B, S, D) view
    tf = np.swapaxes(tgt_out, 0, 1)
    # packed[r] = [l_r | t_r] interleaved
    packed = np.zeros((N_CORES * P * NC + 1, 2, D), np.float32)
    packed[:tot, 0] = lf[bs_b, bs_s]
    packed[:tot, 1] = tf[bs_b, bs_s]

    iso_base = N_CORES * P * RG
    in_maps = []
    for c in range(N_CORES):
        x = np.empty((RG + IG + 1, P, 2, D), np.float32)
        run = packed[c * P * RG:(c + 1) * P * RG].reshape(P, RG, 2, D)
        x[:RG] = run.transpose(1, 0, 2, 3)          # slot-major
        if IG:
            iso = packed[iso_base + c * P * IG:
                         iso_base + (c + 1) * P * IG].reshape(P, IG, 2, D)
            x[RG:RG + IG] = iso.transpose(1, 0, 2, 3)
        # overlap slot: next partition's (or next core's) first run row
        x[RG + IG] = packed[c * P * RG + RG:
                            (c + 1) * P * RG + RG:RG]
        in_maps.append({"x": x})
    kres = run_bass_kernel_spmd(nc, in_maps, list(range(N_CORES)),
                                trace=trace)

    # device columns [run slots | iso slots] -> packed positions
    full = {}
    res = kres.results
    for q, grab in (("ll", lambda a: a["act"][:, :NC]),
                    ("tt", lambda a: a["act"][:, NC:]),
                    ("lt", lambda a: a["dve"][:, :NC])):
        runp = np.concatenate([grab(res[c])[:, :RG].reshape(P * RG)
                               for c in range(N_CORES)])
        isop = (np.concatenate([grab(res[c])[:, RG:].reshape(P * IG)
                                for c in range(N_CORES)]) if IG else
                np.zeros(0, np.float32))
        full[q] = np.concatenate([runp, isop])
    for q, sl in (("ln", slice(NC, NC + RG)), ("tn", slice(NC + RG, None))):
        full[q] = np.concatenate([res[c]["dve"][:, sl].reshape(P * RG)
                                  for c in range(N_CORES)])
    return full, kres


def _finish_host(rows, mask):
    """Host-side float64 finish: reproduce reference semantics exactly."""
    ll = rows["ll"].astype(np.float64)
    tt = rows["tt"].astype(np.float64)
    lt = rows["lt"].astype(np.float64)
    ln = rows["ln"].astype(np.float64)
    tn = rows["tn"].astype(np.float64)

    valid = ~mask                     # (B, S)
    n_valid = float(valid.sum())

    # masked MSE: sum over valid rows of sum_d (l-t)^2 = ll - 2lt + tt
    mse = ((ll - 2.0 * lt + tt) * valid).sum() / (n_valid * D)

    # CosineEmbeddingLoss part (eps = 1e-8)
    na = np.maximum(np.sqrt(ll), 1e-8)
    nb = np.maximum(np.sqrt(tt), 1e-8)
    c = lt / (na * nb)
    loss_cos = ((1.0 - c) * valid).sum() / n_valid

    # consecutive-sentence cosine deltas (eps = 1e-6), shape (B, S-1)
    nl = np.maximum(np.sqrt(ll), 1e-6)
    nt = np.maximum(np.sqrt(tt), 1e-6)
    d_l = ln[:, :S - 1] / (nl[:, :-1] * nl[:, 1:])
    d_t = tn[:, :S - 1] / (nt[:, :-1] * nt[:, 1:])
    pair_valid = valid[:, :-1] & valid[:, 1:]
    cnt = int(pair_valid.sum())
    loss_delta = (np.square(d_l - d_t) * pair_valid).sum() / max(cnt, 1)

    # delta-of-delta on the compacted (valid-only, batch-major) delta lists
    L = B * (S - 1)
    pvf = pair_valid.reshape(-1)

    def dd(d_flat):
        dense = np.zeros(L, np.float64)
        dense[:cnt] = d_flat[pvf]
        prev = dense[:-1]
        den = np.where(prev != 0, prev, 1e-6)
        return (dense[1:] - prev) / den

    dd_l = dd(d_l.reshape(-1))
    dd_t = dd(d_t.reshape(-1))
    dd_valid = np.arange(L - 1) < (cnt - 1)
    n_dd = float(max(cnt - 1, 1))
    loss_dd = (np.square(dd_l - dd_t) * dd_valid).sum() / n_dd / 100.0

    return mse + loss_cos + loss_delta + loss_dd


def kernel(logits, tgt_out, tgt_padding_mask, _trace=False):
    logits = np.asarray(logits, dtype=np.float32)
    tgt_out = np.asarray(tgt_out, dtype=np.float32)
    mask = np.asarray(tgt_padding_mask).astype(bool)

    plan = _plan(mask)
    bs_b, bs_s, pair_ok, RG, IG = plan
    tot = len(bs_b)
    packed, kres = _run_device(logits, tgt_out, plan, trace=_trace)

    # scatter packed results back to full (B, S) arrays; untouched
    # positions stay 0 and are masked out in the finish.
    rows = {}
    for q in ("ll", "tt", "lt"):
        f = np.zeros((B, S), np.float32)
        f[bs_b, bs_s] = packed[q][:tot]
        rows[q] = f
    gok = np.flatnonzero(pair_ok)     # all true pairs live in the run region
    for q in ("ln", "tn"):
        f = np.zeros((B, S), np.float32)
        f[bs_b[gok], bs_s[gok]] = packed[q][gok]
        rows[q] = f

    total = _finish_host(rows, mask)
    out = np.array(total, dtype=np.float32)
    if _trace:
        return out, kres
    return out


# revision 18
# speedup vs baseline: 1.0276x; 1.0276x over previous
"""Trainium2 Bass kernel for nn_Mixture_Loss_74053826118054.

Strategy (valid-row compaction + data parallel over 8 cores):
  Every term of the loss depends only on 5 per-(s,b)-row reductions over D:
    ll = sum_d l^2,  tt = sum_d t^2,  lt = sum_d l*t,
    ln = sum_d l[s]*l[s+1]  (consecutive sentences, same batch),
    tn = sum_d t[s]*t[s+1]
  The padding mask is known on the host, and every quantity is only ever
  USED on valid rows (~50%): masked MSE / cos / deltas all multiply by the
  valid mask, and ln/tn are only used on valid PAIRS (both rows valid,
  ~25%). The host packs only the valid rows, batch-major with maximal
  consecutive-valid runs kept contiguous, and ships the packed array:
  half the HBM traffic and half the compute of the dense kernel.

  Rows in runs of length >= 2 ("run region") need all five quantities;
  isolated valid rows ("iso region", no valid neighbor) need only
  ll/tt/lt. The packed list is [run rows | iso rows]; overflow iso rows
  spill into the run region's spare slots (their pair products are
  garbage the host ignores). Pair validity is re-derived on the host
  from the packed (b, s) list, so any garbage pair (run boundaries,
  spilled iso rows, zero padding) is dropped exactly.

Device layout per core: partitions hold RG consecutive packed rows
(+1 overlap slot = next partition's/core's first row, so consecutive-row
products are free-axis slices) plus IG iso rows. l and t of one row are
interleaved in DRAM ([row, {l,t}, D]) so each chunk load is one 8 KB
descriptor per partition. Per chunk: ACT does the two squares with fused
accumulate; DVE does lt/ln/tn as scalar_tensor_tensor with fused
accumulate. No GpSimd: its SBUF port is shared with DVE ("POOL slot")
and concurrent GpSimd ops double DVE 2-port op latency.

DMA schedule: all chunk loads are issued back-to-back on the Sync
engine's HWDGE ring; one ring = FIFO completion, so a single monotonic
semaphore (16 incs per load) gates the compute pipeline. The overlap
slot is NOT re-read from HBM: ACT's ring copies it SBUF->SBUF with a
one-partition shift (plus one 8 KB HBM edge row for partition 127).
Each compute engine stores its own merged result block on its own ring
right after draining, so the only cross-engine sync is the final
out_sem wait. The tiny O(S*B) finish (cos, deltas, rank-compaction,
delta-of-delta) runs on host in float64, reproducing the reference
semantics exactly.
"""

import numpy as np

from contextlib import ExitStack

import concourse.bass as bass
import concourse.mybir as mybir
from concourse.bass_utils import run_bass_kernel_spmd

F32 = mybir.dt.float32
AF = mybir.ActivationFunctionType
ALU = mybir.AluOpType

N_CORES = 8
S, B, D = 64, 256, 1024
P = 128

_cached = {}


def _build_program(RG, IG):
    """Bass program for RG run-chunks (+1 overlap slot) and IG iso-chunks."""
    key = (RG, IG)
    if key in _cached:
        return _cached[key]
    NSLOT = RG + IG + 1          # run slots + iso slots + overlap (last)
    OV = RG + IG                 # overlap slot index
    NC = RG + IG                 # result columns for ll/tt/lt
    nc = bass.Bass()
    # slot-major, exactly the SBUF layout: x[slot, partition] = [l | t]
    # of that slot's row -> every chunk load is one contiguous 1MB region
    x_in = nc.dram_tensor("x", [NSLOT, P, 2, D], F32, kind="ExternalInput")
    # merged outputs: one store per compute engine
    out_act = nc.dram_tensor("act", [P, 2 * NC], F32, kind="ExternalOutput")
    out_dve = nc.dram_tensor("dve", [P, NC + 2 * RG], F32,
                             kind="ExternalOutput")

    with ExitStack() as stack:
        ec = stack.enter_context
        # one semaphore per chunk: DMA completions are only FIFO per SDMA
        # engine, so a summed counter can pass with a mix of chunks
        csem = [ec(nc.semaphore(f"c{i}")) for i in range(NSLOT)]
        out_sem = ec(nc.semaphore("out"))
        dve_done = ec(nc.semaphore("dvedone"))
        xbig = ec(nc.sbuf_tensor([P, NSLOT * 2 * D], F32))
        dummies = ec(nc.sbuf_tensor([P, 8], F32))
        ract = ec(nc.sbuf_tensor([P, 2 * NC], F32))       # [ll | tt]
        rdve = ec(nc.sbuf_tensor([P, NC + 2 * RG], F32))  # [lt | ln | tn]
        block = ec(nc.Block())
        xc = xbig.ap().rearrange("p (c v d) -> p c v d", v=2, d=D)

        def chunk(slot, half):
            return xc[:, slot, half, :]

        def bcast(k):
            return dummies.ap()[:, k:k + 1].broadcast_to((P, D))

        rll = ract.ap()[:, 0:NC]
        rtt = ract.ap()[:, NC:2 * NC]
        rlt = rdve.ap()[:, 0:NC]
        rln = rdve.ap()[:, NC:NC + RG]
        rtn = rdve.ap()[:, NC + RG:NC + 2 * RG]

        @block.sync
        def _(sync):
            # even slots on the SP ring (odd slots ride the ACT ring in
            # parallel): the two HWDGE rings drain concurrently, so the
            # first two chunks land together ~one chunk-time earlier
            for s in range(0, NSLOT, 2):
                sync.dma_start(out=xc[:, s, :, :],
                               in_=x_in[s]).then_inc(csem[s], 16)
            sync.wait_ge(dve_done, 1)
            sync.dma_start(out=out_dve[:], in_=rdve.ap()).then_inc(
                out_sem, 16)
            sync.wait_ge(out_sem, 32)

        @block.scalar
        def _(scalar):
            for s in range(1, NSLOT, 2):
                scalar.dma_start(out=xc[:, s, :, :],
                                 in_=x_in[s]).then_inc(csem[s], 16)
            for col in range(NC):        # run slots then iso slots = cols
                scalar.wait_ge(csem[col], 16)
                scalar.activation(bcast(0), chunk(col, 0), AF.Square,
                                  accum_out=rll[:, col:col + 1])
                scalar.activation(bcast(1), chunk(col, 1), AF.Square,
                                  accum_out=rtt[:, col:col + 1])
            scalar.drain()
            scalar.dma_start(out=out_act[:], in_=ract.ap()).then_inc(
                out_sem, 16)

        @block.vector
        def _(vector):
            for j in range(RG):
                vector.wait_ge(csem[j], 16)
                vector.scalar_tensor_tensor(
                    out=bcast(2), in0=chunk(j, 0), scalar=0.0,
                    in1=chunk(j, 1), op0=ALU.bypass, op1=ALU.mult,
                    accum_out=rlt[:, j:j + 1])
                if j < RG - 1:           # j = RG-1 pairs with the overlap
                    vector.wait_ge(csem[j + 1], 16)   # slot, loaded last
                    vector.scalar_tensor_tensor(
                        out=bcast(3), in0=chunk(j, 0), scalar=0.0,
                        in1=chunk(j + 1, 0), op0=ALU.bypass, op1=ALU.mult,
                        accum_out=rln[:, j:j + 1])
                    vector.scalar_tensor_tensor(
                        out=bcast(4), in0=chunk(j, 1), scalar=0.0,
                        in1=chunk(j + 1, 1), op0=ALU.bypass, op1=ALU.mult,
                        accum_out=rtn[:, j:j + 1])
            for k in range(IG):
                slot = RG + k
                vector.wait_ge(csem[slot], 16)
                vector.scalar_tensor_tensor(
                    out=bcast(2), in0=chunk(slot, 0), scalar=0.0,
                    in1=chunk(slot, 1), op0=ALU.bypass, op1=ALU.mult,
                    accum_out=rlt[:, slot:slot + 1])
            vector.wait_ge(csem[OV], 16)
            vector.scalar_tensor_tensor(
                out=bcast(3), in0=chunk(RG - 1, 0), scalar=0.0,
                in1=chunk(OV, 0), op0=ALU.bypass, op1=ALU.mult,
                accum_out=rln[:, RG - 1:RG])
            vector.scalar_tensor_tensor(
                out=bcast(4), in0=chunk(RG - 1, 1), scalar=0.0,
                in1=chunk(OV, 1), op0=ALU.bypass, op1=ALU.mult,
                accum_out=rtn[:, RG - 1:RG])
            vector.drain().then_inc(dve_done, 1)

    _cached[key] = nc
    return nc


def _plan(mask):
    """Packed-row plan from the padding mask.

    Returns (bs_b, bs_s, pair_ok, RG, IG): packed order = all rows of
    runs (length >= 2, batch-major, runs contiguous) then isolated valid
    rows. pair_ok[g] marks packed-adjacent pairs (g, g+1) that are true
    consecutive same-batch valid pairs.
    """
    valid = ~mask                                   # (B, S)
    left = np.zeros_like(valid)
    left[:, 1:] = valid[:, :-1]
    right = np.zeros_like(valid)
    right[:, :-1] = valid[:, 1:]
    iso = valid & ~left & ~right
    runm = valid & ~iso
    rb, rs = np.nonzero(runm)                       # lexicographic: runs stay
    ib, is_ = np.nonzero(iso)                       # contiguous in order
    bs_b = np.concatenate([rb, ib])
    bs_s = np.concatenate([rs, is_])
    n_run = len(rb)
    tot = len(bs_b)
    pair_ok = (bs_b[:-1] == bs_b[1:]) & (bs_s[:-1] + 1 == bs_s[1:])
    RG = max(1, -(-n_run // (N_CORES * P)))
    IG = max(0, -(-(tot - N_CORES * P * RG) // (N_CORES * P)))
    return bs_b, bs_s, pair_ok, RG, IG


def _run_device(logits, tgt_out, plan, trace=False):
    bs_b, bs_s, pair_ok, RG, IG = plan
    nc = _build_program(RG, IG)
    NC = RG + IG
    tot = len(bs_b)

    lf = np.swapaxes(logits, 0, 1)                  # (B, S, D) view
    tf = np.swapaxes(tgt_out, 0, 1)
    # packed[r] = [l_r | t_r] interleaved
    packed = np.zeros((N_CORES * P * NC + 1, 2, D), np.float32)
    packed[:tot, 0] = lf[bs_b, bs_s]
    packed[:tot, 1] = tf[bs_b, bs_s]

    iso_base = N_CORES * P * RG
    in_maps = []
    for c in range(N_CORES):
        x = np.empty((RG + IG + 1, P, 2, D), np.float32)
        run = packed[c * P * RG:(c + 1) * P * RG].reshape(P, RG, 2, D)
        x[:RG] = run.transpose(1, 0, 2, 3)          # slot-major
        if IG:
            iso = packed[iso_base + c * P * IG:
                         iso_base + (c + 1) * P * IG].reshape(P, IG, 2, D)
            x[RG:RG + IG] = iso.transpose(1, 0, 2, 3)
        # overlap slot: next partition's (or next core's) first run row
        x[RG + IG] = packed[c * P * RG + RG:
                            (c + 1) * P * RG + RG:RG]
        in_maps.append({"x": x})
    kres = run_bass_kernel_spmd(nc, in_maps, list(range(N_CORES)),
                                trace=trace)

    # device columns [run slots | iso slots] -> packed positions
    full = {}
    res = kres.results
    for q, grab in (("ll", lambda a: a["act"][:, :NC]),
                    ("tt", lambda a: a["act"][:, NC:]),
                    ("lt", lambda a: a["dve"][:, :NC])):
        runp = np.concatenate([grab(res[c])[:, :RG].reshape(P * RG)
                               for c in range(N_CORES)])
        isop = (np.concatenate([grab(res[c])[:, RG:].reshape(P * IG)
                                for c in range(N_CORES)]) if IG else
                np.zeros(0, np.float32))
        full[q] = np.concatenate([runp, isop])
    for q, sl in (("ln", slice(NC, NC + RG)), ("tn", slice(NC + RG, None))):
        full[q] = np.concatenate([res[c]["dve"][:, sl].reshape(P * RG)
                                  for c in range(N_CORES)])
    return full, kres


def _finish_host(rows, mask):
    """Host-side float64 finish: reproduce reference semantics exactly."""
    ll = rows["ll"].astype(np.float64)
    tt = rows["tt"].astype(np.float64)
    lt = rows["lt"].astype(np.float64)
    ln = rows["ln"].astype(np.float64)
    tn = rows["tn"].astype(np.float64)

    valid = ~mask                     # (B, S)
    n_valid = float(valid.sum())

    # masked MSE: sum over valid rows of sum_d (l-t)^2 = ll - 2lt + tt
    mse = ((ll - 2.0 * lt + tt) * valid).sum() / (n_valid * D)

    # CosineEmbeddingLoss part (eps = 1e-8)
    na = np.maximum(np.sqrt(ll), 1e-8)
    nb = np.maximum(np.sqrt(tt), 1e-8)
    c = lt / (na * nb)
    loss_cos = ((1.0 - c) * valid).sum() / n_valid

    # consecutive-sentence cosine deltas (eps = 1e-6), shape (B, S-1)
    nl = np.maximum(np.sqrt(ll), 1e-6)
    nt = np.maximum(np.sqrt(tt), 1e-6)
    d_l = ln[:, :S - 1] / (nl[:, :-1] * nl[:, 1:])
    d_t = tn[:, :S - 1] / (nt[:, :-1] * nt[:, 1:])
    pair_valid = valid[:, :-1] & valid[:, 1:]
    cnt = int(pair_valid.sum())
    loss_delta = (np.square(d_l - d_t) * pair_valid).sum() / max(cnt, 1)

    # delta-of-delta on the compacted (valid-only, batch-major) delta lists
    L = B * (S - 1)
    pvf = pair_valid.reshape(-1)

    def dd(d_flat):
        dense = np.zeros(L, np.float64)
        dense[:cnt] = d_flat[pvf]
        prev = dense[:-1]
        den = np.where(prev != 0, prev, 1e-6)
        return (dense[1:] - prev) / den

    dd_l = dd(d_l.reshape(-1))
    dd_t = dd(d_t.reshape(-1))
    dd_valid = np.arange(L - 1) < (cnt - 1)
    n_dd = float(max(cnt - 1, 1))
    loss_dd = (np.square(dd_l - dd_t) * dd_valid).sum() / n_dd / 100.0

    return mse + loss_cos + loss_delta + loss_dd


def kernel(logits, tgt_out, tgt_padding_mask, _trace=False):
    logits = np.asarray(logits, dtype=np.float32)
    tgt_out = np.asarray(tgt_out, dtype=np.float32)
    mask = np.asarray(tgt_padding_mask).astype(bool)

    plan = _plan(mask)
    bs_b, bs_s, pair_ok, RG, IG = plan
    tot = len(bs_b)
    packed, kres = _run_device(logits, tgt_out, plan, trace=_trace)

    # scatter packed results back to full (B, S) arrays; untouched
    # positions stay 0 and are masked out in the finish.
    rows = {}
    for q in ("ll", "tt", "lt"):
        f = np.zeros((B, S), np.float32)
        f[bs_b, bs_s] = packed[q][:tot]
        rows[q] = f
    gok = np.flatnonzero(pair_ok)     # all true pairs live in the run region
    for q in ("ln", "tn"):
        f = np.zeros((B, S), np.float32)
        f[bs_b[gok], bs_s[gok]] = packed[q][gok]
        rows[q] = f

    total = _finish_host(rows, mask)
    out = np.array(total, dtype=np.float32)
    if _trace:
        return out, kres
    return out
